# revision 7
# baseline (speedup 1.0000x reference)
"""Trainium2 Bass kernel for CRF logZ (nn_CRFModel).

Math: logZ[b] = logsumexp over tag paths of sum(A-transitions) + sum(B-emissions).
Computed in probability space with a constant per-step rescale folded into the
transition matrix (expAs = exp(A - log 64)), which keeps the recurrence state
p~ = exp(alpha - t*log64) bounded in ~[1e-5, 1e-1] so no per-step normalization
is needed.  Per core (data-parallel over 32 sentences):
  1. indirect-DMA gather of the 4096 needed E rows (2KB each, t-major order)
  2. on-chip PE transpose + GEMM  emis[tag, word] = ThetaB @ Erows^T
  3. exp on ScalarE
  4. 128-step recurrence: q = expAs^T @ p (PE), p' = q * expE_t (DVE)
  5. logZ = log(expAs[:,EOS]^T @ p_T) + 129*log(64)
Masking: expAs[:, BOS]=0 (no transition into BOS), expAs[EOS, :]=0 (nothing
leaves EOS) and the final contraction uses expAs[:, EOS] whose EOS entry is 0 -
together exactly equivalent to the reference's NEG masking of A and B.
"""

import sys

for _p in ("/opt/trn_rl_repo", "/root/.axon_site/_ro/trn_rl_repo"):
    if _p not in sys.path:
        sys.path.insert(0, _p)

import math

import numpy as np

import concourse.bass as bass
import concourse.mybir as mybir
import concourse.tile as tile
from concourse import bacc
from concourse.bass_utils import run_bass_kernel_spmd
from concourse.masks import make_identity

K = 64
V = 50257
D = 512
BT = 256
T = 128
BOS = 62
EOS = 63
N_CORES = 8
B_PER_CORE = BT // N_CORES          # 32 sentences per core
W_PER_CORE = B_PER_CORE * T         # 4096 gathered words per core
N_GTILES = W_PER_CORE // 128        # 32 gather tiles of 128 rows
LOG64 = math.log(64.0)

F32 = mybir.dt.float32
I32 = mybir.dt.int32

_CACHE = {}


def _build():
    nc = bacc.Bacc("TRN2", target_bir_lowering=False, debug=False,
                   num_devices=N_CORES)

    idx_d = nc.dram_tensor("idx", [128, N_GTILES], I32, kind="ExternalInput").ap()
    wa_d = nc.dram_tensor("WA", [K, K], F32, kind="ExternalInput").ap()
    th_d = nc.dram_tensor("ThetaB", [K, D], F32, kind="ExternalInput").ap()
    e_d = nc.dram_tensor("E", [V, D], F32, kind="ExternalInput").ap()
    out_d = nc.dram_tensor("out", [1, B_PER_CORE], F32, kind="ExternalOutput").ap()

    with tile.TileContext(nc) as tc:
        with (
            tc.tile_pool(name="const", bufs=1) as cpool,
            tc.tile_pool(name="erows", bufs=3) as erpool,
            tc.tile_pool(name="erowsT", bufs=3) as etpool,
            tc.tile_pool(name="pst", bufs=2) as ppool,
            tc.tile_pool(name="psum_tr", bufs=2, space="PSUM") as ps_tr,
            tc.tile_pool(name="psum_em", bufs=2, space="PSUM") as ps_em,
            tc.tile_pool(name="psum_q", bufs=3, space="PSUM") as ps_q,
            tc.tile_pool(name="psum_z", bufs=1, space="PSUM") as ps_z,
        ):
            # ---- constants ------------------------------------------------
            iden = cpool.tile([128, 128], F32, tag="iden")
            make_identity(nc, iden[:])

            idx_sb = cpool.tile([128, N_GTILES], I32, tag="idx")
            nc.sync.dma_start(idx_sb[:], idx_d[:])

            wa_sb = cpool.tile([K, K], F32, tag="wa")
            nc.sync.dma_start(wa_sb[:], wa_d[:])

            # expAs = exp(WA - log64); col BOS = 0; row EOS = 0
            nlog64 = cpool.tile([K, 1], F32, tag="nlog64")
            nc.vector.memset(nlog64[:], -LOG64)
            expas = cpool.tile([K, K], F32, tag="expas")
            nc.scalar.activation(expas[:], wa_sb[:],
                                 mybir.ActivationFunctionType.Exp,
                                 bias=nlog64[:], scale=1.0)
            nc.vector.memset(expas[:, BOS:BOS + 1], 0.0)
            # zero row EOS: keep where (x - EOS) != 0, else fill 0
            nc.gpsimd.affine_select(
                out=expas[:], in_=expas[:],
                compare_op=mybir.AluOpType.not_equal,
                fill=0.0, base=-EOS, pattern=[[0, K]], channel_multiplier=1)

            # ThetaB^T in 4 chunks of [128, 64]
            th_sb = cpool.tile([K, D], F32, tag="th")
            nc.sync.dma_start(th_sb[:], th_d[:])
            thT = []
            for c in range(4):
                ps = ps_tr.tile([128, 128], F32, tag="er_ps")
                nc.tensor.transpose(ps[:, 0:K], th_sb[:, c * 128:(c + 1) * 128],
                                    iden[0:K, 0:K])
                t_sb = cpool.tile([128, K], F32, tag=f"thT{c}")
                nc.vector.tensor_copy(t_sb[:], ps[:, 0:K])
                thT.append(t_sb)

            # initial state p0 = one-hot(BOS) for all 32 sentences
            p_cur = ppool.tile([K, B_PER_CORE], F32, tag="p")
            nc.vector.memset(p_cur[:], 0.0)
            nc.gpsimd.affine_select(
                out=p_cur[:], in_=p_cur[:],
                compare_op=mybir.AluOpType.not_equal,
                fill=1.0, base=-BOS, pattern=[[0, B_PER_CORE]],
                channel_multiplier=1)

            # ---- pipeline: gather -> transpose -> GEMM -> exp -> 4 steps --
            for g in range(N_GTILES):
                er = erpool.tile([128, D], F32, tag="er")
                nc.gpsimd.indirect_dma_start(
                    out=er[:], out_offset=None, in_=e_d[:],
                    in_offset=bass.IndirectOffsetOnAxis(
                        ap=idx_sb[:, g:g + 1], axis=0),
                )
                erT = etpool.tile([128, D], F32, tag="erT")
                for c in range(4):
                    ps = ps_tr.tile([128, 128], F32, tag="er_ps")
                    nc.tensor.transpose(ps[:], er[:, c * 128:(c + 1) * 128],
                                        iden[:])
                    nc.vector.tensor_copy(erT[:, c * 128:(c + 1) * 128], ps[:])

                em_ps = ps_em.tile([K, 128], F32, tag="em")
                for c in range(4):
                    nc.tensor.matmul(em_ps[:], lhsT=thT[c][:],
                                     rhs=erT[:, c * 128:(c + 1) * 128],
                                     start=(c == 0), stop=(c == 3))
                expe = cpool.tile([K, 128], F32, tag=f"expe{g}")
                nc.scalar.activation(expe[:], em_ps[:],
                                     mybir.ActivationFunctionType.Exp)

                for tt in range(4):
                    q = ps_q.tile([K, B_PER_CORE], F32, tag="q")
                    nc.tensor.matmul(q[:], lhsT=expas[:], rhs=p_cur[:],
                                     start=True, stop=True)
                    p_nxt = ppool.tile([K, B_PER_CORE], F32, tag="p")
                    nc.vector.tensor_mul(
                        p_nxt[:], q[:],
                        expe[:, tt * B_PER_CORE:(tt + 1) * B_PER_CORE])
                    p_cur = p_nxt

            # ---- finale ---------------------------------------------------
            z = ps_z.tile([1, B_PER_CORE], F32, tag="z")
            nc.tensor.matmul(z[:], lhsT=expas[:, EOS:EOS + 1], rhs=p_cur[:],
                             start=True, stop=True)
            lnz = cpool.tile([1, B_PER_CORE], F32, tag="lnz")
            nc.scalar.activation(lnz[:], z[:], mybir.ActivationFunctionType.Ln)
            res = cpool.tile([1, B_PER_CORE], F32, tag="res")
            nc.vector.tensor_scalar_add(res[:], lnz[:], float((T + 1) * LOG64))
            nc.sync.dma_start(out_d[:], res[:])

    nc.compile()
    return nc


def _get_nc():
    if "nc" not in _CACHE:
        _CACHE["nc"] = _build()
    return _CACHE["nc"]


def kernel(words, WA, ThetaB, E):
    words = np.asarray(words)
    WA = np.ascontiguousarray(np.asarray(WA, np.float32))
    ThetaB = np.ascontiguousarray(np.asarray(ThetaB, np.float32))
    E = np.ascontiguousarray(np.asarray(E, np.float32))

    nc = _get_nc()
    in_maps = []
    for c in range(N_CORES):
        wb = words[c * B_PER_CORE:(c + 1) * B_PER_CORE].astype(np.int32)  # [32,128]
        wf = wb.T.reshape(-1)                    # t-major flat: j = t*32 + b
        idx = np.ascontiguousarray(wf.reshape(N_GTILES, 128).T)  # [128, 32]
        in_maps.append({"idx": idx, "WA": WA, "ThetaB": ThetaB, "E": E})

    res = run_bass_kernel_spmd(nc, in_maps, list(range(N_CORES)))
    return np.concatenate(
        [res.results[c]["out"][0] for c in range(N_CORES)]).astype(np.float32)


# revision 12
# speedup vs baseline: 1.1769x; 1.1769x over previous
"""Trainium2 Bass kernel for CRF logZ (nn_CRFModel).

Math: probability-space forward recurrence with a constant per-step rescale
folded into the transitions (expAs = exp(A - log64)); the state
p~ = exp(alpha - t*log64) stays in ~[1e-5, 1e-1] so no per-step
normalization is needed.  logZ = log(expAs[:,EOS]^T p~_T) + 129*log64.

Per core (data-parallel, 32 sentences each):
  1. xbar dma_gather(transpose=True) pulls the 4096 needed E rows (fp16)
     from two half-vocab tables (int16 index limit) directly in
     D-on-partitions layout: out[p, c, w] = E[word_w, 128c+p].
  2. copy_predicated merges the two gathers (hi-vocab words overwrite).
  3. GEMM emis[tag, w] = ThetaB @ Erows^T in fp16, N=512 per matmul.
  4. exp on ScalarE -> expE.
  5. 128-step recurrence split into two 16-sentence chains, phase-
     interleaved so PE/DVE semaphore latency of one chain hides under the
     other's work: q = expAs^T p (PE, fp16), p' = q * expE_t (DVE).
Masking: expAs[:, BOS]=0, expAs[EOS, :]=0, and the final contraction
column has EOS entry 0 - exactly equivalent to the reference's NEG masks.
"""

import sys

for _p in ("/opt/trn_rl_repo", "/root/.axon_site/_ro/trn_rl_repo"):
    if _p not in sys.path:
        sys.path.insert(0, _p)

import math

import numpy as np

import concourse.bass as bass
import concourse.mybir as mybir
import concourse.tile as tile
from concourse import bacc
from concourse.bass_utils import run_bass_kernel_spmd

K = 64
V = 50257
D = 512
BT = 256
T = 128
BOS = 62
EOS = 63
N_CORES = 8
B_PER_CORE = BT // N_CORES          # 32 sentences per core
HB = B_PER_CORE // 2                # 16 sentences per chain
W_PER_CORE = B_PER_CORE * T         # 4096 gathered words per core
VSPLIT = 32768                      # int16 index limit
NW_G = 512                          # words per gather instruction
N_G = W_PER_CORE // NW_G            # 8 gather groups
T_G = T // N_G                      # 16 time steps per group
LOG64 = math.log(64.0)

F32 = mybir.dt.float32
F16 = mybir.dt.float16
I16 = mybir.dt.int16
U8 = mybir.dt.uint8

_CACHE = {}


def _build():
    nc = bacc.Bacc("TRN2", target_bir_lowering=False, debug=False,
                   num_devices=N_CORES)

    S = W_PER_CORE // 16  # 256 idx slots per partition-row
    ilo_d = nc.dram_tensor("idxlo", [128, S], I16, kind="ExternalInput").ap()
    ihi_d = nc.dram_tensor("idxhi", [128, S], I16, kind="ExternalInput").ap()
    msk_d = nc.dram_tensor("maskhi", [128, 4 * W_PER_CORE], U8,
                           kind="ExternalInput").ap()
    wa_d = nc.dram_tensor("WA", [K, K], F32, kind="ExternalInput").ap()
    th_d = nc.dram_tensor("ThetaB", [K, D], F32, kind="ExternalInput").ap()
    elo_d = nc.dram_tensor("Elo", [VSPLIT, D], F16, kind="ExternalInput").ap()
    ehi_d = nc.dram_tensor("Ehi", [V - VSPLIT, D], F16,
                           kind="ExternalInput").ap()
    out_d = nc.dram_tensor("out", [1, B_PER_CORE], F32,
                           kind="ExternalOutput").ap()

    with tile.TileContext(nc) as tc:
        with (
            tc.tile_pool(name="const", bufs=1) as cpool,
            tc.tile_pool(name="gat", bufs=2) as gpool,
            tc.tile_pool(name="pst", bufs=2) as ppool,
            tc.tile_pool(name="psum_tr", bufs=1, space="PSUM") as ps_tr,
            tc.tile_pool(name="psum_em", bufs=2, space="PSUM") as ps_em,
            tc.tile_pool(name="psum_qa", bufs=2, space="PSUM") as ps_qa,
            tc.tile_pool(name="psum_qb", bufs=2, space="PSUM") as ps_qb,
            tc.tile_pool(name="psum_z", bufs=1, space="PSUM") as ps_z,
        ):
            # ---- constants ------------------------------------------------
            ilo = cpool.tile([128, S], I16, tag="ilo")
            nc.sync.dma_start(ilo[:], ilo_d[:])
            ihi = cpool.tile([128, S], I16, tag="ihi")
            nc.sync.dma_start(ihi[:], ihi_d[:])
            msk = cpool.tile([128, 4 * W_PER_CORE], U8, tag="msk")
            nc.sync.dma_start(msk[:], msk_d[:])

            wa_sb = cpool.tile([K, K], F32, tag="wa")
            nc.sync.dma_start(wa_sb[:], wa_d[:])

            # expAs = exp(WA - log64); col BOS = 0; row EOS = 0
            nlog64 = cpool.tile([K, 1], F32, tag="nlog64")
            nc.vector.memset(nlog64[:], -LOG64)
            expas = cpool.tile([K, K], F32, tag="expas")
            nc.scalar.activation(expas[:], wa_sb[:],
                                 mybir.ActivationFunctionType.Exp,
                                 bias=nlog64[:], scale=1.0)
            nc.vector.memset(expas[:, BOS:BOS + 1], 0.0)
            nc.gpsimd.affine_select(
                out=expas[:], in_=expas[:],
                compare_op=mybir.AluOpType.not_equal,
                fill=0.0, base=-EOS, pattern=[[0, K]], channel_multiplier=1)
            expas_bf = cpool.tile([K, K], F16, tag="expas_bf")
            nc.vector.tensor_copy(expas_bf[:], expas[:])

            # ThetaB^T in 4 fp16 chunks of [128, 64] (one-time, fp32 PE
            # transpose then cast on the PSUM->SBUF copy)
            th_sb = cpool.tile([K, D], F32, tag="th")
            nc.sync.dma_start(th_sb[:], th_d[:])
            iden = cpool.tile([K, K], F32, tag="iden")
            nc.gpsimd.memset(iden[:], 0.0)
            nc.gpsimd.affine_select(
                out=iden[:], in_=iden[:],
                compare_op=mybir.AluOpType.not_equal,
                fill=1.0, base=0, pattern=[[-1, K]], channel_multiplier=1)
            thT = []
            for c in range(4):
                ps = ps_tr.tile([128, K], F32, tag="thT_ps")
                nc.tensor.transpose(ps[:], th_sb[:, c * 128:(c + 1) * 128],
                                    iden[:])
                t_bf = cpool.tile([128, K], F16, tag=f"thT{c}")
                nc.vector.tensor_copy(t_bf[:], ps[:])
                thT.append(t_bf)

            # initial state p0 = one-hot(BOS), two half-batch chains
            pA = ppool.tile([K, HB], F16, tag="pA")
            nc.vector.memset(pA[:], 0.0)
            nc.gpsimd.affine_select(
                out=pA[:], in_=pA[:], compare_op=mybir.AluOpType.not_equal,
                fill=1.0, base=-BOS, pattern=[[0, HB]], channel_multiplier=1)
            pB = ppool.tile([K, HB], F16, tag="pB")
            nc.vector.tensor_copy(pB[:], pA[:])

            # ---- pipeline over 8 groups of 512 words (16 steps each) ------
            for g in range(N_G):
                sl = slice(g * (NW_G // 16), (g + 1) * (NW_G // 16))
                glo = gpool.tile([128, 4 * NW_G], F16, tag="glo")
                nc.gpsimd.dma_gather(
                    glo[:].rearrange("p (c w) -> p c w", c=4),
                    elo_d[:], ilo[:, sl], NW_G, NW_G, D, transpose=True)
                ghi = gpool.tile([128, 4 * NW_G], F16, tag="ghi")
                nc.gpsimd.dma_gather(
                    ghi[:].rearrange("p (c w) -> p c w", c=4),
                    ehi_d[:], ihi[:, sl], NW_G, NW_G, D, transpose=True)
                nc.vector.copy_predicated(
                    glo[:], msk[:, g * 4 * NW_G:(g + 1) * 4 * NW_G], ghi[:])

                em_ps = ps_em.tile([K, NW_G], F32, tag="em")
                for c in range(4):
                    nc.tensor.matmul(em_ps[:], lhsT=thT[c][:],
                                     rhs=glo[:, c * NW_G:(c + 1) * NW_G],
                                     start=(c == 0), stop=(c == 3))
                expe = cpool.tile([K, NW_G], F32, tag=f"expe{g}")
                nc.scalar.activation(expe[:], em_ps[:],
                                     mybir.ActivationFunctionType.Exp)

                for tt in range(T_G):
                    w0 = tt * B_PER_CORE
                    qa = ps_qa.tile([K, HB], F32, tag="qa")
                    nc.tensor.matmul(qa[:], lhsT=expas_bf[:], rhs=pA[:],
                                     start=True, stop=True)
                    qb = ps_qb.tile([K, HB], F32, tag="qb")
                    nc.tensor.matmul(qb[:], lhsT=expas_bf[:], rhs=pB[:],
                                     start=True, stop=True)
                    pA = ppool.tile([K, HB], F16, tag="pA")
                    nc.vector.tensor_mul(pA[:], qa[:],
                                         expe[:, w0:w0 + HB])
                    pB = ppool.tile([K, HB], F16, tag="pB")
                    nc.vector.tensor_mul(pB[:], qb[:],
                                         expe[:, w0 + HB:w0 + B_PER_CORE])

            # ---- finale ---------------------------------------------------
            z = ps_z.tile([1, B_PER_CORE], F32, tag="z")
            nc.tensor.matmul(z[:, 0:HB], lhsT=expas_bf[:, EOS:EOS + 1],
                             rhs=pA[:], start=True, stop=True)
            nc.tensor.matmul(z[:, HB:B_PER_CORE],
                             lhsT=expas_bf[:, EOS:EOS + 1],
                             rhs=pB[:], start=True, stop=True)
            lnz = cpool.tile([1, B_PER_CORE], F32, tag="lnz")
            nc.scalar.activation(lnz[:], z[:], mybir.ActivationFunctionType.Ln)
            res = cpool.tile([1, B_PER_CORE], F32, tag="res")
            nc.vector.tensor_scalar_add(res[:], lnz[:], float((T + 1) * LOG64))
            nc.sync.dma_start(out_d[:], res[:])

    nc.compile()
    return nc


def _get_nc():
    if "nc" not in _CACHE:
        _CACHE["nc"] = _build()
    return _CACHE["nc"]


def _wrap16(w):
    """idx j -> partition j%16, slot j//16; replicated to all 8 Q7 cores."""
    a = np.asarray(w, np.int16).reshape(-1, 16).T  # [16, S]
    return np.tile(a, (8, 1))                      # [128, S]


def _make_in_maps(words, WA, ThetaB, E):
    words = np.asarray(words)
    WA = np.ascontiguousarray(np.asarray(WA, np.float32))
    ThetaB = np.ascontiguousarray(np.asarray(ThetaB, np.float32))
    E = np.asarray(E, np.float32)
    Elo = np.ascontiguousarray(E[:VSPLIT].astype(np.float16))
    Ehi = np.ascontiguousarray(E[VSPLIT:].astype(np.float16))

    in_maps = []
    for c in range(N_CORES):
        wb = words[c * B_PER_CORE:(c + 1) * B_PER_CORE].astype(np.int64)
        wf = wb.T.reshape(-1)                    # t-major flat: j = t*32 + b
        is_hi = wf >= VSPLIT
        wlo = np.where(is_hi, 0, wf).astype(np.int16)
        whi = np.where(is_hi, wf - VSPLIT, 0).astype(np.int16)
        m = np.tile(is_hi.reshape(N_G, NW_G), (1, 4)).reshape(-1)
        mask = np.repeat(m.astype(np.uint8)[None, :], 128, axis=0)
        in_maps.append({
            "idxlo": _wrap16(wlo), "idxhi": _wrap16(whi),
            "maskhi": np.ascontiguousarray(mask),
            "WA": WA, "ThetaB": ThetaB, "Elo": Elo, "Ehi": Ehi,
        })
    return in_maps


def kernel(words, WA, ThetaB, E):
    nc = _get_nc()
    in_maps = _make_in_maps(words, WA, ThetaB, E)
    res = run_bass_kernel_spmd(nc, in_maps, list(range(N_CORES)))
    return np.concatenate(
        [res.results[c]["out"][0] for c in range(N_CORES)]).astype(np.float32)


# revision 15
# speedup vs baseline: 1.4840x; 1.2610x over previous
"""Trainium2 Bass kernel for CRF logZ (nn_CRFModel).

Math: probability-space forward recurrence with a constant per-step rescale
folded into the transitions (expAs = exp(A - log64)); the state
p~ = exp(alpha - t*log64) stays in ~[1e-5, 1e-1] so no per-step
normalization is needed.  logZ = log(expAs[:,EOS]^T p~_T) + 129*log64.

Per core (data-parallel, 32 sentences each):
  1. xbar dma_gather(transpose=True) pulls the 4096 needed E rows (fp16)
     from two half-vocab tables (int16 index limit) directly in
     D-on-partitions layout: out[p, c, w] = E[word_w, 128c+p].
  2. copy_predicated merges the two gathers (hi-vocab words overwrite).
  3. GEMM emis[tag, w] = ThetaB @ Erows^T in fp16, N=512 per matmul.
  4. exp on ScalarE -> expE.
  5. 128-step recurrence split into two 16-sentence chains, phase-
     interleaved so PE/DVE semaphore latency of one chain hides under the
     other's work: q = expAs^T p (PE, fp16), p' = q * expE_t (DVE).
Masking: expAs[:, BOS]=0, expAs[EOS, :]=0, and the final contraction
column has EOS entry 0 - exactly equivalent to the reference's NEG masks.
"""

import sys

for _p in ("/opt/trn_rl_repo", "/root/.axon_site/_ro/trn_rl_repo"):
    if _p not in sys.path:
        sys.path.insert(0, _p)

import math

import numpy as np

import concourse.bass as bass
import concourse.mybir as mybir
import concourse.tile as tile
from concourse import bacc
from concourse.bass_utils import run_bass_kernel_spmd
from concourse.tile import add_dep_helper

K = 64
V = 50257
D = 512
BT = 256
T = 128
BOS = 62
EOS = 63
N_CORES = 8
B_PER_CORE = BT // N_CORES          # 32 sentences per core
HB = B_PER_CORE // 2                # 16 sentences per chain
W_PER_CORE = B_PER_CORE * T         # 4096 gathered words per core
VSPLIT = 32768                      # int16 index limit
NW_G = 512                          # words per gather instruction
N_G = W_PER_CORE // NW_G            # 8 gather groups
T_G = T // N_G                      # 16 time steps per group
LOG64 = math.log(64.0)

F32 = mybir.dt.float32
F16 = mybir.dt.float16
I16 = mybir.dt.int16
U8 = mybir.dt.uint8

_CACHE = {}


def _build():
    nc = bacc.Bacc("TRN2", target_bir_lowering=False, debug=False,
                   num_devices=N_CORES)

    S = W_PER_CORE // 16  # 256 idx slots per partition-row
    ilo_d = nc.dram_tensor("idxlo", [128, S], I16, kind="ExternalInput").ap()
    ihi_d = nc.dram_tensor("idxhi", [128, S], I16, kind="ExternalInput").ap()
    msk_d = nc.dram_tensor("maskhi", [128, 4 * W_PER_CORE], U8,
                           kind="ExternalInput").ap()
    wa_d = nc.dram_tensor("WA", [K, K], F32, kind="ExternalInput").ap()
    th_d = nc.dram_tensor("ThetaB", [K, D], F32, kind="ExternalInput").ap()
    elo_d = nc.dram_tensor("Elo", [VSPLIT, D], F16, kind="ExternalInput").ap()
    ehi_d = nc.dram_tensor("Ehi", [V - VSPLIT, D], F16,
                           kind="ExternalInput").ap()
    out_d = nc.dram_tensor("out", [1, B_PER_CORE], F32,
                           kind="ExternalOutput").ap()

    with tile.TileContext(nc) as tc:
        with (
            tc.tile_pool(name="const", bufs=1) as cpool,
            tc.tile_pool(name="gat", bufs=4) as gpool,
            tc.tile_pool(name="pst", bufs=2) as ppool,
            tc.tile_pool(name="psum_tr", bufs=1, space="PSUM") as ps_tr,
            tc.tile_pool(name="psum_em", bufs=2, space="PSUM") as ps_em,
            tc.tile_pool(name="psum_qa", bufs=2, space="PSUM") as ps_qa,
            tc.tile_pool(name="psum_qb", bufs=2, space="PSUM") as ps_qb,
            tc.tile_pool(name="psum_z", bufs=1, space="PSUM") as ps_z,
        ):
            # ---- constants ------------------------------------------------
            ilo = cpool.tile([128, S], I16, tag="ilo")
            nc.sync.dma_start(ilo[:], ilo_d[:])
            ihi = cpool.tile([128, S], I16, tag="ihi")
            nc.sync.dma_start(ihi[:], ihi_d[:])
            msk = cpool.tile([128, 4 * W_PER_CORE], U8, tag="msk")
            nc.sync.dma_start(msk[:], msk_d[:])

            wa_sb = cpool.tile([K, K], F32, tag="wa")
            nc.sync.dma_start(wa_sb[:], wa_d[:])

            # expAs = exp(WA - log64); col BOS = 0; row EOS = 0
            nlog64 = cpool.tile([K, 1], F32, tag="nlog64")
            nc.vector.memset(nlog64[:], -LOG64)
            expas = cpool.tile([K, K], F32, tag="expas")
            nc.scalar.activation(expas[:], wa_sb[:],
                                 mybir.ActivationFunctionType.Exp,
                                 bias=nlog64[:], scale=1.0)
            nc.vector.memset(expas[:, BOS:BOS + 1], 0.0)
            nc.gpsimd.affine_select(
                out=expas[:], in_=expas[:],
                compare_op=mybir.AluOpType.not_equal,
                fill=0.0, base=-EOS, pattern=[[0, K]], channel_multiplier=1)
            expas_bf = cpool.tile([K, K], F16, tag="expas_bf")
            nc.vector.tensor_copy(expas_bf[:], expas[:])

            # ThetaB^T in 4 fp16 chunks of [128, 64] (one-time, fp32 PE
            # transpose then cast on the PSUM->SBUF copy)
            th_sb = cpool.tile([K, D], F32, tag="th")
            nc.sync.dma_start(th_sb[:], th_d[:])
            iden = cpool.tile([K, K], F32, tag="iden")
            nc.gpsimd.memset(iden[:], 0.0)
            nc.gpsimd.affine_select(
                out=iden[:], in_=iden[:],
                compare_op=mybir.AluOpType.not_equal,
                fill=1.0, base=0, pattern=[[-1, K]], channel_multiplier=1)
            thT = []
            for c in range(4):
                ps = ps_tr.tile([128, K], F32, tag="thT_ps")
                nc.tensor.transpose(ps[:], th_sb[:, c * 128:(c + 1) * 128],
                                    iden[:])
                t_bf = cpool.tile([128, K], F16, tag=f"thT{c}")
                nc.vector.tensor_copy(t_bf[:], ps[:])
                thT.append(t_bf)

            # initial state p0 = one-hot(BOS), two half-batch chains
            pA = ppool.tile([K, HB], F16, tag="pA")
            nc.vector.memset(pA[:], 0.0)
            nc.gpsimd.affine_select(
                out=pA[:], in_=pA[:], compare_op=mybir.AluOpType.not_equal,
                fill=1.0, base=-BOS, pattern=[[0, HB]], channel_multiplier=1)
            pB = ppool.tile([K, HB], F16, tag="pB")
            nc.vector.tensor_copy(pB[:], pA[:])

            # ---- pipeline over 8 groups of 512 words (16 steps each) ------
            # Order-only anchors so the scheduler interleaves each group's
            # emission work into the previous group's recurrence instead of
            # running the whole emission phase first (PE/DVE are FIFO).
            rec_mm = []   # recurrence matmul instructions of previous group
            rec_mul = []  # recurrence multiply instructions of previous group
            for g in range(N_G):
                sl = slice(g * (NW_G // 16), (g + 1) * (NW_G // 16))
                glo = gpool.tile([128, 4 * NW_G], F16, tag="glo")
                nc.gpsimd.dma_gather(
                    glo[:].rearrange("p (c w) -> p c w", c=4),
                    elo_d[:], ilo[:, sl], NW_G, NW_G, D, transpose=True)
                ghi = gpool.tile([128, 4 * NW_G], F16, tag="ghi")
                nc.gpsimd.dma_gather(
                    ghi[:].rearrange("p (c w) -> p c w", c=4),
                    ehi_d[:], ihi[:, sl], NW_G, NW_G, D, transpose=True)
                mrg = nc.vector.copy_predicated(
                    glo[:], msk[:, g * 4 * NW_G:(g + 1) * 4 * NW_G], ghi[:])
                if rec_mul:
                    add_dep_helper(mrg.ins, rec_mul[8].ins,
                                   reason="interleave merge into prev recurrence")

                em_ps = ps_em.tile([K, NW_G], F32, tag="em")
                for c in range(4):
                    mm = nc.tensor.matmul(em_ps[:], lhsT=thT[c][:],
                                          rhs=glo[:, c * NW_G:(c + 1) * NW_G],
                                          start=(c == 0), stop=(c == 3))
                    if rec_mm and c == 0:
                        add_dep_helper(mm.ins, rec_mm[20].ins,
                                       reason="interleave gemm into prev recurrence")
                expe = cpool.tile([K, NW_G], F32, tag=f"expe{g}")
                nc.scalar.activation(expe[:], em_ps[:],
                                     mybir.ActivationFunctionType.Exp)

                rec_mm, rec_mul = [], []
                for tt in range(T_G):
                    w0 = tt * B_PER_CORE
                    qa = ps_qa.tile([K, HB], F32, tag="qa")
                    rec_mm.append(
                        nc.tensor.matmul(qa[:], lhsT=expas_bf[:], rhs=pA[:],
                                         start=True, stop=True))
                    qb = ps_qb.tile([K, HB], F32, tag="qb")
                    rec_mm.append(
                        nc.tensor.matmul(qb[:], lhsT=expas_bf[:], rhs=pB[:],
                                         start=True, stop=True))
                    pA = ppool.tile([K, HB], F16, tag="pA")
                    rec_mul.append(
                        nc.vector.tensor_mul(pA[:], qa[:],
                                             expe[:, w0:w0 + HB]))
                    pB = ppool.tile([K, HB], F16, tag="pB")
                    rec_mul.append(
                        nc.vector.tensor_mul(pB[:], qb[:],
                                             expe[:, w0 + HB:w0 + B_PER_CORE]))

            # ---- finale ---------------------------------------------------
            z = ps_z.tile([1, B_PER_CORE], F32, tag="z")
            nc.tensor.matmul(z[:, 0:HB], lhsT=expas_bf[:, EOS:EOS + 1],
                             rhs=pA[:], start=True, stop=True)
            nc.tensor.matmul(z[:, HB:B_PER_CORE],
                             lhsT=expas_bf[:, EOS:EOS + 1],
                             rhs=pB[:], start=True, stop=True)
            lnz = cpool.tile([1, B_PER_CORE], F32, tag="lnz")
            nc.scalar.activation(lnz[:], z[:], mybir.ActivationFunctionType.Ln)
            res = cpool.tile([1, B_PER_CORE], F32, tag="res")
            nc.vector.tensor_scalar_add(res[:], lnz[:], float((T + 1) * LOG64))
            nc.sync.dma_start(out_d[:], res[:])

    nc.compile()
    return nc


def _get_nc():
    if "nc" not in _CACHE:
        _CACHE["nc"] = _build()
    return _CACHE["nc"]


def _wrap16(w):
    """idx j -> partition j%16, slot j//16; replicated to all 8 Q7 cores."""
    a = np.asarray(w, np.int16).reshape(-1, 16).T  # [16, S]
    return np.tile(a, (8, 1))                      # [128, S]


def _make_in_maps(words, WA, ThetaB, E):
    words = np.asarray(words)
    WA = np.ascontiguousarray(np.asarray(WA, np.float32))
    ThetaB = np.ascontiguousarray(np.asarray(ThetaB, np.float32))
    E = np.asarray(E, np.float32)
    Elo = np.ascontiguousarray(E[:VSPLIT].astype(np.float16))
    Ehi = np.ascontiguousarray(E[VSPLIT:].astype(np.float16))

    in_maps = []
    for c in range(N_CORES):
        wb = words[c * B_PER_CORE:(c + 1) * B_PER_CORE].astype(np.int64)
        wf = wb.T.reshape(-1)                    # t-major flat: j = t*32 + b
        is_hi = wf >= VSPLIT
        wlo = np.where(is_hi, 0, wf).astype(np.int16)
        whi = np.where(is_hi, wf - VSPLIT, 0).astype(np.int16)
        m = np.tile(is_hi.reshape(N_G, NW_G), (1, 4)).reshape(-1)
        mask = np.repeat(m.astype(np.uint8)[None, :], 128, axis=0)
        in_maps.append({
            "idxlo": _wrap16(wlo), "idxhi": _wrap16(whi),
            "maskhi": np.ascontiguousarray(mask),
            "WA": WA, "ThetaB": ThetaB, "Elo": Elo, "Ehi": Ehi,
        })
    return in_maps


def kernel(words, WA, ThetaB, E):
    nc = _get_nc()
    in_maps = _make_in_maps(words, WA, ThetaB, E)
    res = run_bass_kernel_spmd(nc, in_maps, list(range(N_CORES)))
    return np.concatenate(
        [res.results[c]["out"][0] for c in range(N_CORES)]).astype(np.float32)
